# revision 14
# baseline (speedup 1.0000x reference)
"""Trainium2 Bass kernel for a GNN message-passing decoder layer.

Reference computation (N=4096 nodes, K=48 neighbors, H=128, E_IN=384):
  h_EV = concat([broadcast(h_V), h_E], -1)          # [N, K, 512]
  h = gelu(h_EV @ W1 + b1); h = gelu(h @ W2 + b2)   # per-edge MLP
  msg = h @ W3 + b3
  dh = sum_k(mask_attend * msg) / 30
  x1 = LN1(h_V + dh)
  dh2 = gelu(x1 @ Win + bin) @ Wout + bout
  out = mask_V * LN2(x1 + dh2)

Sharding: node dimension split across 8 NeuronCores (512 nodes/core), weights
replicated.  Per core, nodes are processed in 4 blocks of 128; edge tokens are
laid out k-major (token = k*128 + n).

mask_attend handling (valid for 0/1 masks): the mask is multiplied into h_E
on the host, so a masked token's MLP input is the unmasked h_V part only; its
message is then exactly gamma_n = MLP(h_V-only token), a per-node constant.
The device computes the plain k-sum of messages and subtracts
cnt0_n * (W3^T gamma_n), where cnt0 = K - sum_k mask comes from two tiny PE
matmuls; the whole gamma path is batched over all 512 nodes of the core.  For
the all-ones benchmark mask the correction is exactly zero but always
computed.

Precision: the edge-token stream (masked h_E + replicated h_V) and W1 run in
fp8 e4m3 (weights pre-scaled by 16, undone by the activation scale), packed in
DoubleRow pairs so W1 runs at 0.5 PE cycles/row.  g1/g2 and W2/Win/Wout are
bf16; the k-reduction is 2 contiguous bf16 adds on DVE (2x mode);
residual/LayerNorm/W3 stay fp32.  Measured rel err vs fp32 reference ~2.6e-4.

Engine-level choices driven by timeline-sim profiling:
 - DMA issue cost dominates small transfers, so the edge stream moves in 8
   half-block DMAs of 1.5MB, constants are packed into a few DMAs by dtype,
   and the output is staged in SBUF and written with a single DMA.
 - The Activation engine is the critical resource (2 gelus per edge token);
   j-steps are processed in pairs so each gelu instruction covers 1024
   columns, halving per-instruction overhead.  PSUM tiles are [128,2,512]
   (two banks), psA/psB x2 bufs = all 8 banks.
 - LayerNorm rstd uses a Quake-style Newton rsqrt on DVE (batched over the 4
   blocks) instead of the Activation-engine Sqrt: sqrt is in a different
   activation-function set than gelu, and each use forced a ~1.3us
   LoadActFuncSet table reload (~13us/iteration across 10 reloads).
"""

import os
import sys

sys.path.insert(0, "/opt/trn_rl_repo")

import numpy as np

N, K, H, E_IN = 4096, 48, 128, 384
NCORES = 8
NPC = N // NCORES          # nodes per core = 512
NBLK = NPC // 128          # node blocks per core = 4
TPB = K * 128              # tokens per block = 6144
NJ = TPB // 512            # j-steps per block = 12
SCALE = 30.0
EPS = 1e-5
WS = 16.0                  # fp8 weight pre-scale (undone via activation scale)
QMAGIC = 0x5F3759DF

_CACHE = {}


def _build_nc(reps=1, **_ignored):
    import concourse.bass as bass
    import concourse.mybir as mybir
    from concourse import bacc
    from concourse.bass import ts
    from concourse.tile import TileContext
    from contextlib import ExitStack

    F32 = mybir.dt.float32
    BF16 = mybir.dt.bfloat16
    FP8 = mybir.dt.float8e4
    U8 = mybir.dt.uint8
    U32 = mybir.dt.uint32
    GELU = mybir.ActivationFunctionType.Gelu
    DR = mybir.MatmulPerfMode.DoubleRow
    MULT = mybir.AluOpType.mult
    SUB = mybir.AluOpType.subtract
    SHR = mybir.AluOpType.logical_shift_right

    nc = bacc.Bacc()

    # edge stream: per block, NJ j-steps of [128, 2048] = pairA | pairB
    xe = nc.dram_tensor("xe", [NBLK, 128, NJ * 2048], U8, kind="ExternalInput")
    # packed constants by dtype
    pk8 = nc.dram_tensor("pk8", [128, 1024], U8, kind="ExternalInput")
    pkb = nc.dram_tensor("pkb", [128, 1152], BF16, kind="ExternalInput")
    pkf = nc.dram_tensor("pkf", [128, 647], F32, kind="ExternalInput")
    b3x48 = nc.dram_tensor("b3x48", [K, H], F32, kind="ExternalInput")
    w3f = nc.dram_tensor("w3f", [H, H], F32, kind="ExternalInput")
    hvtM = nc.dram_tensor("hvtM", [128, NBLK * H], F32, kind="ExternalInput")
    m48M = nc.dram_tensor("m48M", [K, NBLK * 128], F32, kind="ExternalInput")
    mvM = nc.dram_tensor("mvM", [128, NBLK], F32, kind="ExternalInput")
    out = nc.dram_tensor("out", [NPC, H], F32, kind="ExternalOutput")

    with TileContext(nc) as tc, ExitStack() as ctx:
        const = ctx.enter_context(tc.tile_pool(name="const", bufs=1))
        xep = ctx.enter_context(tc.tile_pool(name="xe", bufs=2))
        g1p = ctx.enter_context(tc.tile_pool(name="g1", bufs=3))
        g2p = ctx.enter_context(tc.tile_pool(name="g2", bufs=3))
        rtp = ctx.enter_context(tc.tile_pool(name="rt", bufs=3))
        rbp = ctx.enter_context(tc.tile_pool(name="rb", bufs=2))
        gmp = ctx.enter_context(tc.tile_pool(name="gm", bufs=2))
        crp = ctx.enter_context(tc.tile_pool(name="cr", bufs=2))
        smp = ctx.enter_context(tc.tile_pool(name="sm", bufs=3))
        xqp = ctx.enter_context(tc.tile_pool(name="xq", bufs=5))
        x1p = ctx.enter_context(tc.tile_pool(name="x1", bufs=5))
        qp = ctx.enter_context(tc.tile_pool(name="qp", bufs=2))
        psA = ctx.enter_context(tc.tile_pool(name="psA", bufs=2, space="PSUM"))
        psB = ctx.enter_context(tc.tile_pool(name="psB", bufs=2, space="PSUM"))

        pk8_t = const.tile([128, 1024], FP8, tag="pk8")
        nc.sync.dma_start(out=pk8_t, in_=pk8[:].bitcast(FP8))
        pkb_t = const.tile([128, 1152], BF16, tag="pkb")
        nc.sync.dma_start(out=pkb_t, in_=pkb[:])
        pkf_t = const.tile([128, 647], F32, tag="pkf")
        nc.sync.dma_start(out=pkf_t, in_=pkf[:])
        b3x48_t = const.tile([K, H], F32, tag="b3x48")
        nc.sync.dma_start(out=b3x48_t, in_=b3x48[:])
        w3_t = const.tile([H, H], F32, tag="w3")
        nc.sync.dma_start(out=w3_t, in_=w3f[:])
        hvtM_t = const.tile([128, NBLK * H], F32, tag="hvtM")
        nc.sync.dma_start(out=hvtM_t, in_=hvtM[:])
        m48M_t = const.tile([K, NBLK * 128], F32, tag="m48M")
        nc.sync.dma_start(out=m48M_t, in_=m48M[:])
        mvb_t = const.tile([128, NBLK], F32, tag="mvb")
        nc.sync.dma_start(out=mvb_t, in_=mvM[:])

        # packed-constant views
        w1aP = pk8_t[:, 0:256].rearrange("p (i m) -> p i m", i=2)
        w1bP = pk8_t[:, 256:512].rearrange("p (i m) -> p i m", i=2)
        hvf_t = pk8_t[:, 512:1024]            # [H, NPC] fp8 h_V feature-major
        w2_t = pkb_t[:, 0:128]
        win_t = pkb_t[:, 128:640]
        wout_t = pkb_t[:, 640:1152]
        ident_t = pkf_t[:, 0:128]
        s1b_t = pkf_t[:, 128:256]
        o1b_t = pkf_t[:, 256:384]
        s2b_t = pkf_t[:, 384:512]
        o2b_t = pkf_t[:, 512:640]
        binc_t = pkf_t[:, 640:644]
        b1_t = pkf_t[:, 644:645]
        b2_t = pkf_t[:, 645:646]
        bout_t = pkf_t[:, 646:647]

        x1F = const.tile([H, NPC], BF16, tag="x1F")
        z_sb = const.tile([128, 4, NPC], BF16, tag="z_sb")
        ystage = const.tile([128, NBLK, H], F32, tag="ystage")
        mvs1 = const.tile([128, NBLK, 2], F32, tag="mvs1")
        mvs2 = const.tile([128, NBLK, 2], F32, tag="mvs2")
        negones_t = const.tile([K, H], F32, tag="negones")
        nc.vector.memset(negones_t, -1.0)
        kvec_t = const.tile([1, H], F32, tag="kvec")
        nc.vector.memset(kvec_t, float(K))
        onesr_t = const.tile([1, NPC], F32, tag="onesr")
        nc.vector.memset(onesr_t, 1.0)
        one_u = const.tile([128, NBLK], U32, tag="one_u")
        nc.vector.memset(one_u, 1)
        magic_u = const.tile([128, NBLK], U32, tag="magic_u")
        nc.vector.memset(magic_u, QMAGIC)

        ISCALE = 1.0 / WS

        def quake_rstd(var_ap, w):
            """rstd[128, w] = 1/sqrt(var + EPS) on DVE only (Quake + 2 NR)."""
            ve = qp.tile([128, w], F32, tag=f"ve{w}")
            nc.vector.tensor_scalar_add(out=ve[:], in0=var_ap, scalar1=EPS)
            sh = qp.tile([128, w], U32, tag=f"sh{w}")
            nc.vector.tensor_tensor(
                out=sh[:], in0=ve[:].bitcast(U32), in1=one_u[:, 0:w], op=SHR
            )
            seed = qp.tile([128, w], F32, tag=f"seed{w}")
            nc.vector.tensor_tensor(
                out=seed[:].bitcast(U32), in0=magic_u[:, 0:w], in1=sh[:], op=SUB
            )
            y = seed
            for it in range(2):
                a = qp.tile([128, w], F32, tag=f"qa{it}{w}")
                nc.vector.tensor_tensor(out=a[:], in0=y[:], in1=y[:], op=MULT)
                nc.vector.tensor_tensor(out=a[:], in0=a[:], in1=ve[:], op=MULT)
                nc.vector.tensor_scalar(
                    out=a[:], in0=a[:], scalar1=-0.5, scalar2=1.5,
                    op0=MULT, op1=mybir.AluOpType.add,
                )
                yn = qp.tile([128, w], F32, tag=f"qy{it}{w}")
                nc.vector.tensor_tensor(out=yn[:], in0=y[:], in1=a[:], op=MULT)
                y = yn
            return y

        for _rep in range(reps):
            # --- gamma path, batched over all 512 nodes -------------------
            psu = psA.tile([128, 2, 512], F32, tag="ps1")
            nc.tensor.matmul(psu[:, 0, :], w1bP[:, 1, :], hvf_t, start=True, stop=True)
            g1g = gmp.tile([H, NPC], BF16, tag="g1g")
            nc.scalar.activation(
                out=g1g[:], in_=psu[:, 0, :], func=GELU, bias=b1_t, scale=ISCALE
            )
            psv = psB.tile([128, 2, 512], F32, tag="ps2")
            nc.tensor.matmul(psv[:, 0, :], w2_t, g1g[:], start=True, stop=True)
            g2g = gmp.tile([H, NPC], F32, tag="g2g")
            nc.scalar.activation(out=g2g[:], in_=psv[:, 0, :], func=GELU, bias=b2_t)
            psw = psA.tile([128, 2, 512], F32, tag="ps1")
            nc.tensor.matmul(psw[:, 0, :], w3_t[:], g2g[:], start=True, stop=True)
            psb_ = psB.tile([128, 2, 512], F32, tag="ps2")
            nc.tensor.matmul(psb_[:, 0, :], negones_t[:], m48M_t[:], start=True, stop=False)
            nc.tensor.matmul(psb_[:, 0, :], kvec_t[:], onesr_t[:], start=False, stop=True)
            psi_sb = crp.tile([H, NPC], F32, tag="psi")
            nc.vector.tensor_copy(out=psi_sb[:], in_=psw[:, 0, :])
            corr_all = crp.tile([H, NPC], F32, tag="corr")
            nc.vector.tensor_mul(out=corr_all[:], in0=psi_sb[:], in1=psb_[:, 0, :])

            # --- edge MLP main loop --------------------------------------
            x1_tiles = []
            for b in range(NBLK):
                rblk = rbp.tile([H, 128], F32, tag="rblk")

                def reduce_pair(ps2, idx):
                    # gelu2 (one pair behind the W2 matmuls, so the in-order
                    # Activation queue never stalls on W2 latency) + k-reduce
                    g2 = g2p.tile([H, 2, 512], BF16, tag="g2")
                    nc.scalar.activation(out=g2[:], in_=ps2[:], func=GELU, bias=b2_t)
                    t12 = rtp.tile([H, 2, 256], BF16, tag="t12")
                    nc.vector.tensor_add(
                        out=t12[:], in0=g2[:, :, 0:256], in1=g2[:, :, 256:512]
                    )
                    rp = rtp.tile([H, 2, 128], BF16, tag="rp")
                    nc.vector.tensor_add(
                        out=rp[:], in0=t12[:, :, 0:128], in1=t12[:, :, 128:256]
                    )
                    if idx == 0:
                        nc.vector.tensor_add(
                            out=rblk[:], in0=rp[:, 0, :], in1=rp[:, 1, :]
                        )
                    else:
                        nc.gpsimd.tensor_add(out=rblk[:], in0=rblk[:], in1=rp[:, 0, :])
                        nc.gpsimd.tensor_add(out=rblk[:], in0=rblk[:], in1=rp[:, 1, :])

                pend = None
                for half in range(2):
                    tx = xep.tile([128, 6, 2048], FP8, tag="tx")
                    nc.sync.dma_start(
                        out=tx,
                        in_=xe[b, :, half * 12288 : (half + 1) * 12288]
                        .rearrange("p (j t) -> p j t", j=6)
                        .bitcast(FP8),
                    )
                    for p2 in range(3):
                        ps1 = psA.tile([128, 2, 512], F32, tag="ps1")
                        for i in range(2):
                            jj = p2 * 2 + i
                            nc.tensor.matmul(
                                ps1[:, i, :], w1aP,
                                tx[:, jj, 0:1024].rearrange("p (i t) -> p i t", i=2),
                                start=True, stop=False, perf_mode=DR,
                            )
                            nc.tensor.matmul(
                                ps1[:, i, :], w1bP,
                                tx[:, jj, 1024:2048].rearrange("p (i t) -> p i t", i=2),
                                start=False, stop=True, perf_mode=DR,
                            )
                        g1 = g1p.tile([H, 2, 512], BF16, tag="g1")
                        nc.scalar.activation(
                            out=g1[:], in_=ps1[:], func=GELU, bias=b1_t, scale=ISCALE
                        )
                        ps2 = psB.tile([128, 2, 512], F32, tag="ps2")
                        for i in range(2):
                            nc.tensor.matmul(
                                ps2[:, i, :], w2_t, g1[:, i, :], start=True, stop=True
                            )
                        if pend is not None:
                            reduce_pair(*pend)
                        pend = (ps2, half * 3 + p2)
                reduce_pair(*pend)

                # block tail: message, residual, LN1 (per-block DVE rsqrt)
                psd = psB.tile([128, 2, 512], F32, tag="ps2")
                nc.tensor.matmul(psd[:, 0, 0:128], w3_t[:], rblk[:], start=True, stop=False)
                nc.tensor.matmul(
                    psd[:, 0, 0:128], b3x48_t[:], m48M_t[:, ts(b, 128)],
                    start=False, stop=True,
                )
                dh_sb = smp.tile([H, 128], F32, tag="dh_sb")
                nc.vector.tensor_sub(
                    out=dh_sb[:], in0=psd[:, 0, 0:128], in1=corr_all[:, ts(b, 128)]
                )
                pst = psA.tile([128, 2, 512], F32, tag="ps1")
                nc.tensor.transpose(pst[:, 0, 0:128], dh_sb[:], ident_t)
                x1pre = xqp.tile([128, H], F32, tag="x1pre")
                nc.vector.tensor_add(out=x1pre[:], in0=hvtM_t[:, ts(b, H)], in1=pst[:, 0, 0:128])
                st6 = smp.tile([128, 6], F32, tag="st6")
                nc.vector.bn_stats(out=st6[:], in_=x1pre[:])
                nc.vector.bn_aggr(out=mvs1[:, b, :], in_=st6[:])
                rstd1 = quake_rstd(mvs1[:, b, 1:2], 1)
                xn = smp.tile([128, H], F32, tag="xn")
                nc.vector.tensor_scalar(
                    out=xn[:], in0=x1pre[:],
                    scalar1=mvs1[:, b, 0:1], scalar2=rstd1[:, 0:1],
                    op0=SUB, op1=MULT,
                )
                x1 = x1p.tile([128, H], F32, tag="x1")
                nc.vector.tensor_mul(out=x1[:], in0=xn[:], in1=s1b_t)
                nc.vector.tensor_add(out=x1[:], in0=x1[:], in1=o1b_t)
                x1_tiles.append(x1)
                pst2 = psB.tile([128, 2, 512], F32, tag="ps2")
                nc.tensor.transpose(pst2[:, 0, 0:128], x1[:], ident_t)
                nc.vector.tensor_copy(out=x1F[:, ts(b, 128)], in_=pst2[:, 0, 0:128])

            # --- FFN (paired gelus) --------------------------------------
            for cp in range(2):
                psz = psA.tile([128, 2, 512], F32, tag="ps1")
                for i in range(2):
                    c = cp * 2 + i
                    nc.tensor.matmul(
                        psz[:, i, :], win_t[:, ts(c, 128)], x1F[:], start=True, stop=True
                    )
                for i in range(2):
                    c = cp * 2 + i
                    nc.scalar.activation(
                        out=z_sb[:, c, :], in_=psz[:, i, :],
                        func=GELU, bias=binc_t[:, c : c + 1],
                    )
            psd2 = psB.tile([128, 2, 512], F32, tag="ps2")
            for c in range(4):
                nc.tensor.matmul(
                    psd2[:, 0, :], wout_t[:, ts(c, 128)], z_sb[:, c, :],
                    start=(c == 0), stop=(c == 3),
                )

            # --- LN2 (batched rsqrt), masked output, single DMA ----------
            x2_tiles = []
            for b in range(NBLK):
                dh2 = smp.tile([H, 128], F32, tag="dh2")
                nc.vector.tensor_scalar_add(
                    out=dh2[:], in0=psd2[:, 0, ts(b, 128)], scalar1=bout_t
                )
                pst = psA.tile([128, 2, 512], F32, tag="ps1")
                nc.tensor.transpose(pst[:, 0, 0:128], dh2[:], ident_t)
                x2 = xqp.tile([128, H], F32, tag="x1pre")
                nc.vector.tensor_add(out=x2[:], in0=x1_tiles[b][:], in1=pst[:, 0, 0:128])
                x2_tiles.append(x2)
                st6 = smp.tile([128, 6], F32, tag="st6")
                nc.vector.bn_stats(out=st6[:], in_=x2[:])
                nc.vector.bn_aggr(out=mvs2[:, b, :], in_=st6[:])
            rstd2 = quake_rstd(mvs2[:, :, 1], NBLK)
            for b in range(NBLK):
                y = ystage[:, b, :]
                nc.vector.tensor_scalar(
                    out=y, in0=x2_tiles[b][:],
                    scalar1=mvs2[:, b, 0:1], scalar2=rstd2[:, b : b + 1],
                    op0=SUB, op1=MULT,
                )
                nc.vector.tensor_mul(out=y, in0=y, in1=s2b_t)
                nc.vector.tensor_add(out=y, in0=y, in1=o2b_t)
                nc.vector.tensor_scalar_mul(out=y, in0=y, scalar1=mvb_t[:, b : b + 1])
            nc.sync.dma_start(
                out=out[:].rearrange("(b p) h -> p b h", b=NBLK), in_=ystage[:]
            )

    nc.finalize()
    return nc


def _get_nc():
    if "nc" not in _CACHE:
        _CACHE["nc"] = _build_nc()
    return _CACHE["nc"]


XE_BF16 = False  # legacy flag kept for test.py compatibility


def _prep_inputs(h_V, h_E, mask_V, mask_attend, W1_w, W1_b, W2_w, W2_b, W3_w, W3_b,
                 Win_w, Win_b, Wout_w, Wout_b, norm1_s, norm1_o, norm2_s, norm2_o):
    import ml_dtypes

    f = np.float32
    FP8 = ml_dtypes.float8_e4m3
    BF16 = ml_dtypes.bfloat16
    h_V = np.asarray(h_V, f)
    h_E = np.asarray(h_E, f)
    mask_V = np.asarray(mask_V, f)
    mask_attend = np.asarray(mask_attend, f)
    W1 = np.asarray(W1_w, f)

    # host-masked edge features, fp8, k-major tokens: xe8[c, b, f, k*128+n]
    xem = h_E * mask_attend[:, :, None]
    xe8 = np.ascontiguousarray(
        xem.reshape(NCORES, NBLK, 128, K, E_IN).transpose(0, 1, 4, 3, 2)
    ).reshape(NCORES, NBLK, E_IN, TPB).astype(FP8)
    # pair A: feature rows 0..255 -> [c, b, 128p, NJ, 2, 512]
    xeA = np.ascontiguousarray(
        xe8[:, :, 0:256].reshape(NCORES, NBLK, 2, 128, NJ, 512)
        .transpose(0, 1, 3, 4, 2, 5)
    ).reshape(NCORES, NBLK, 128, NJ, 1024)
    # pair B: [h_E rows 256..383 | replicated h_V rows]
    xeB_e = xe8[:, :, 256:384].reshape(NCORES, NBLK, 128, NJ, 512)
    hvf8 = np.ascontiguousarray(
        h_V.reshape(NCORES, NPC, H).transpose(0, 2, 1)
    ).astype(FP8)                                    # [c, H, 512n]
    hvfb = hvf8.reshape(NCORES, H, NBLK, 128).transpose(0, 2, 1, 3)  # [c,b,H,128]
    hvrep = np.broadcast_to(
        hvfb[:, :, :, None, :], (NCORES, NBLK, H, K, 128)
    ).reshape(NCORES, NBLK, H, NJ, 512)
    xeB = np.stack([xeB_e, hvrep], axis=4).reshape(NCORES, NBLK, 128, NJ, 1024)
    # interleave: per j, [pairA(1024) | pairB(1024)]
    xe_all = np.ascontiguousarray(
        np.stack([xeA, xeB], axis=4)                 # [c,b,p,NJ,2,1024]
    ).reshape(NCORES, NBLK, 128, NJ * 2048).view(np.uint8)

    hvtM = np.ascontiguousarray(
        h_V.reshape(NCORES, NBLK, 128, H).transpose(0, 2, 1, 3)
    ).reshape(NCORES, 128, NBLK * H)
    m48M = np.ascontiguousarray(
        mask_attend.reshape(NCORES, NBLK, 128, K).transpose(0, 3, 1, 2)
    ).reshape(NCORES, K, NBLK * 128)
    mvM = np.ascontiguousarray(mask_V.reshape(NCORES, NBLK, 128).transpose(0, 2, 1))

    w1a8 = np.ascontiguousarray(
        np.stack([W1[128:256], W1[256:384]], axis=1) * WS
    ).reshape(128, 256).astype(FP8)
    w1b8 = np.ascontiguousarray(
        np.stack([W1[384:512], W1[0:128]], axis=1) * WS
    ).reshape(128, 256).astype(FP8)

    pkb = np.concatenate([
        np.asarray(W2_w, f).astype(BF16),
        np.asarray(Win_w, f).astype(BF16),
        np.ascontiguousarray(
            np.asarray(Wout_w, f).reshape(4, 128, H).transpose(1, 0, 2)
        ).reshape(128, 4 * H).astype(BF16),
    ], axis=1)

    pkf = np.concatenate([
        np.eye(128, dtype=f),
        np.broadcast_to(np.asarray(norm1_s, f)[None, :], (128, H)),
        np.broadcast_to(np.asarray(norm1_o, f)[None, :], (128, H)),
        np.broadcast_to(np.asarray(norm2_s, f)[None, :], (128, H)),
        np.broadcast_to(np.asarray(norm2_o, f)[None, :], (128, H)),
        np.ascontiguousarray(np.asarray(Win_b, f).reshape(4, 128).T),
        np.broadcast_to(np.asarray(W1_b, f)[:, None], (128, 1)),
        np.broadcast_to(np.asarray(W2_b, f)[:, None], (128, 1)),
        np.broadcast_to(np.asarray(Wout_b, f)[:, None], (128, 1)),
    ], axis=1)
    pkf = np.ascontiguousarray(pkf)

    shared = {
        "pkb": pkb,
        "pkf": pkf,
        "w3f": np.asarray(W3_w, f) / SCALE,
        "b3x48": np.ascontiguousarray(
            np.broadcast_to(np.asarray(W3_b, f)[None, :] / SCALE, (K, H))
        ),
    }
    in_maps = []
    for c in range(NCORES):
        pk8 = np.concatenate(
            [w1a8, w1b8, hvf8[c]], axis=1
        ).view(np.uint8)
        m = {
            "xe": xe_all[c],
            "pk8": np.ascontiguousarray(pk8),
            "hvtM": hvtM[c],
            "m48M": m48M[c],
            "mvM": mvM[c],
        }
        m.update(shared)
        in_maps.append(m)
    return in_maps


def run(trace=False, **inputs):
    from concourse.bass_utils import run_bass_kernel_spmd

    nc = _get_nc()
    in_maps = _prep_inputs(**inputs)
    res = run_bass_kernel_spmd(nc, in_maps, core_ids=list(range(NCORES)), trace=trace)
    outp = np.concatenate([r["out"] for r in res.results], axis=0)
    return outp.astype(np.float32), res


def kernel(**inputs):
    outp, _ = run(trace=False, **inputs)
    return outp
